# revision 40
# baseline (speedup 1.0000x reference)
"""Trainium2 Bass kernel: 3x3 windowed 2D attention layer (LN -> QKV -> win-attn -> out proj).

Sharding: 8 cores = (batch b, H-half s). Each core handles one batch image's
32 query rows with a 1-row halo. LayerNorm is folded into the weights:
  xn = (x - mu) * rstd  (per (b,c) over full HxW)
  q/k/v = xn @ W + b  ==  x @ (rstd*W) - (mu*rstd) @ W + b
The k offset/bias are softmax-invariant (constant shift of every key seen by
a query), the v offset/bias fold into the output bias, and the output
projection folds into V entirely (V' = x @ (rstd * (Wv@Wo)) since row
scaling commutes through left matmul). Only q keeps a small rank-1 runtime
correction. Stats need the full image, so each core also loads the other
half's rows (xr) just for stats.

Attention: per 2-query-row block, logits[128 tok, 264 keys] = one PE matmul
(q block vs 4 key rows in a 66-wide zero-padded layout). An additive mask
(-1e30 off the 9 window diagonals) + ACT exp(accum=Z) implement the 9-way
softmax over the 264-wide row (max subtraction is skipped: |logit| <= ~8 for
standardized inputs). ctx@Wo comes from a PE matmul of transposed E chunks
against token-major V' = x@(rstd*Wv@Wo); output is written token-major and
transposed on the host during the gather.
"""

import ml_dtypes
import numpy as np

P = 128
C = 256
B, H, W = 4, 64, 64
HS = 32          # query rows per core
RW = 34          # rows with halo (1 pad/real row above + below)
WP = W + 2       # zero-padded width
NTOK = RW * WP   # 2244 tokens per core (padded image)
NQ = HS * W      # 2048 query tokens
NBLK = 16        # query row-pair blocks
KT = 4 * WP      # 264 keys per block (4 rows x 66)
NEG = -1.0e30

_CACHE = {}


def _build_nc():
    import concourse.bass as bass  # noqa: F401
    import concourse.mybir as mybir
    import concourse.tile as tile
    from concourse import bacc
    from contextlib import ExitStack

    f32 = mybir.dt.float32
    f32r = mybir.dt.float32r
    Alu = mybir.AluOpType
    Act = mybir.ActivationFunctionType

    nc = bacc.Bacc(None, target_bir_lowering=False, debug=False)

    xd = nc.declare_dram_parameter("x", [C, RW, W], f32, isOutput=False)
    xrd = nc.declare_dram_parameter("xr", [C, HS, W], mybir.dt.bfloat16, isOutput=False)
    wqd = nc.declare_dram_parameter("wq", [C, C], f32, isOutput=False)
    wkd = nc.declare_dram_parameter("wk", [C, C], f32, isOutput=False)
    wvod = nc.declare_dram_parameter("wvo", [C, C], f32, isOutput=False)
    bqd = nc.declare_dram_parameter("bq", [C], f32, isOutput=False)
    bod = nc.declare_dram_parameter("bo", [C], f32, isOutput=False)
    maskd = nc.declare_dram_parameter("mask", [P, KT], f32, isOutput=False)
    pmd = nc.declare_dram_parameter("pm", [P, 2], f32, isOutput=False)
    identd = nc.declare_dram_parameter("ident", [P, P], f32, isOutput=False)
    onesd = nc.declare_dram_parameter("ones", [1, P], f32, isOutput=False)
    yd = nc.declare_dram_parameter("y", [NQ, C], f32, isOutput=True)

    with tile.TileContext(nc) as tc, ExitStack() as es:
        cpool = es.enter_context(tc.tile_pool(name="const", bufs=1))
        xpool = es.enter_context(tc.tile_pool(name="x", bufs=1))
        spool = es.enter_context(tc.tile_pool(name="stat", bufs=1))
        scrpool = es.enter_context(tc.tile_pool(name="scr", bufs=2))
        qkv_pool = es.enter_context(tc.tile_pool(name="qkv", bufs=1))
        apool = es.enter_context(tc.tile_pool(name="attn", bufs=3))
        ypool = es.enter_context(tc.tile_pool(name="y", bufs=3))
        es_mm = ExitStack()
        ps_mm = es_mm.enter_context(tc.tile_pool(name="ps_mm", bufs=2, space="PSUM"))

        def any_copy(on_dve, out, in_):
            if on_dve:
                nc.vector.tensor_copy(out, in_)
            else:
                nc.scalar.activation(out=out, in_=in_, func=Act.Copy)

        # ---- load x (into 66-wide zero-padded layout) ----
        x_sb = []
        for kh in range(2):
            xt = xpool.tile([P, RW, WP], f32r, name=f"x{kh}")
            nc.sync.dma_start(out=xt[:, 0:17, 1:W + 1], in_=xd[kh * P:(kh + 1) * P, 0:17, :].bitcast(f32r))
            nc.scalar.dma_start(out=xt[:, 17:RW, 1:W + 1], in_=xd[kh * P:(kh + 1) * P, 17:RW, :].bitcast(f32r))
            x_sb.append(xt)
        xr_sb = []
        for kh in range(2):
            xrt = xpool.tile([P, HS, W], mybir.dt.bfloat16, name=f"xr{kh}")
            nc.sync.dma_start(out=xrt[:, 0:16, :], in_=xrd[kh * P:(kh + 1) * P, 0:16, :])
            nc.scalar.dma_start(out=xrt[:, 16:HS, :], in_=xrd[kh * P:(kh + 1) * P, 16:HS, :])
            xr_sb.append(xrt)

        # ---- constants ----
        w_sb = {}
        for nm, d in (("q", wqd), ("k", wkd), ("vo", wvod)):
            w_sb[nm] = [cpool.tile([P, C], f32, name=f"w{nm}{kh}") for kh in range(2)]
            for kh in range(2):
                nc.gpsimd.dma_start(out=w_sb[nm][kh][:], in_=d[kh * P:(kh + 1) * P, :])
        mask_sb = cpool.tile([P, KT], f32, name="mask")
        nc.gpsimd.dma_start(out=mask_sb[:], in_=maskd[:])
        ident_sb = cpool.tile([P, P], f32r, name="ident")
        nc.gpsimd.dma_start(out=ident_sb[:], in_=identd[:].bitcast(f32r))
        bq_sb = cpool.tile([P, 2], f32, name="bq")
        nc.gpsimd.dma_start(out=bq_sb[:], in_=bqd.rearrange("(h p) -> p h", p=P))
        bo_sb = cpool.tile([1, C], f32, name="bo")
        nc.gpsimd.dma_start(out=bo_sb[:], in_=bod.rearrange("(a c) -> a c", a=1))
        pm_sb = cpool.tile([P, 2], f32, name="pm")
        nc.gpsimd.dma_start(out=pm_sb[:], in_=pmd[:])

        # ---- layernorm stats over the full image ----
        # sum of squares on ACT (Square + accumulate), sums on DVE, in parallel
        rstd, mrs = [], []
        eps_t = spool.tile([P, 1], f32, name="eps")
        nc.vector.memset(eps_t[:], 1e-5)
        for kh in range(2):
            # own-half partial sums per DMA chunk (overlaps the x load);
            # full-image stats come from a pair-wise AllReduce below
            regs = [x_sb[kh][:, 1:17, 1:W + 1].bitcast(f32),
                    x_sb[kh][:, 17:HS + 1, 1:W + 1].bitcast(f32),
                    xr_sb[kh][:, 0:16, :], xr_sb[kh][:, 16:HS, :]]
            sq_parts, sm_parts = [], []
            for pi, rg in enumerate(regs):
                scr = scrpool.tile([P, 16, W], f32, tag="scr", name=f"scr{kh}{pi}")
                sqp = spool.tile([P, 1], f32, name=f"ssqp{kh}{pi}")
                nc.scalar.activation(out=scr[:], in_=rg, func=Act.Square, accum_out=sqp[:])
                sq_parts.append(sqp)
                smp = spool.tile([P, 1], f32, name=f"smp{kh}{pi}")
                nc.vector.tensor_reduce(out=smp[:], in_=rg, axis=mybir.AxisListType.XY, op=Alu.add)
                sm_parts.append(smp)
            ssq = spool.tile([P, 1], f32, name=f"ssq{kh}")
            nc.vector.tensor_tensor(out=ssq[:], in0=sq_parts[0][:], in1=sq_parts[1][:], op=Alu.add)
            nc.vector.tensor_tensor(out=ssq[:], in0=ssq[:], in1=sq_parts[2][:], op=Alu.add)
            nc.vector.tensor_tensor(out=ssq[:], in0=ssq[:], in1=sq_parts[3][:], op=Alu.add)
            sm = spool.tile([P, 1], f32, name=f"sm{kh}")
            nc.vector.tensor_tensor(out=sm[:], in0=sm_parts[0][:], in1=sm_parts[1][:], op=Alu.add)
            nc.vector.tensor_tensor(out=sm[:], in0=sm[:], in1=sm_parts[2][:], op=Alu.add)
            nc.vector.tensor_tensor(out=sm[:], in0=sm[:], in1=sm_parts[3][:], op=Alu.add)
            mean = spool.tile([P, 1], f32, name=f"mean{kh}")
            nc.vector.tensor_scalar_mul(mean[:], sm[:], 1.0 / (H * W))
            ex2 = spool.tile([P, 1], f32, name=f"ex2{kh}")
            nc.vector.tensor_scalar_mul(ex2[:], ssq[:], 1.0 / (H * W))
            msq = spool.tile([P, 1], f32, name=f"msq{kh}")
            nc.vector.tensor_tensor(out=msq[:], in0=mean[:], in1=mean[:], op=Alu.mult)
            var = spool.tile([P, 1], f32, name=f"var{kh}")
            nc.vector.tensor_tensor(out=var[:], in0=ex2[:], in1=msq[:], op=Alu.subtract)
            std = spool.tile([P, 1], f32, name=f"std{kh}")
            nc.scalar.activation(out=std[:], in_=var[:], func=Act.Sqrt, bias=eps_t[:], scale=1.0)
            rs = spool.tile([P, 1], f32, name=f"rstd{kh}")
            nc.vector.reciprocal(rs[:], std[:])
            rstd.append(rs)
            mr = spool.tile([P, 1], f32, name=f"mrs{kh}")
            nc.vector.tensor_tensor(out=mr[:], in0=mean[:], in1=rs[:], op=Alu.mult)
            mrs.append(mr)

            # Fill pad positions of x with the channel mean so pad tokens
            # behave exactly like xn=0 under the rstd-folded weights (keeps
            # the key offset uniform across a window -> softmax-invariant).
            # The pad row location differs per core; pm selects it (0/1).
            for ri, r in ((0, 0), (1, RW - 1)):
                br = spool.tile([P, 1], f32, name=f"br{kh}{ri}")
                nc.vector.tensor_tensor(out=br[:], in0=mean[:], in1=pm_sb[:, ri:ri + 1], op=Alu.mult)
                nc.scalar.activation(out=x_sb[kh][:, r, :], in_=x_sb[kh][:, r, :].bitcast(f32),
                                     func=Act.Identity, bias=br[:], scale=1.0)
            for c0 in (0, WP - 1):
                nc.scalar.activation(out=x_sb[kh][:, :, c0:c0 + 1],
                                     in_=x_sb[kh][:, :, c0:c0 + 1].bitcast(f32),
                                     func=Act.Identity, bias=mean[:], scale=0.0)

        # ---- scale W rows by rstd:  W'[c, :] = rstd_c * W[c, :] ----
        ws = {}
        for nm in ("q", "k", "vo"):
            ws[nm] = []
            for kh in range(2):
                t = cpool.tile([P, C], f32r, name=f"ws{nm}{kh}")
                nc.vector.tensor_scalar_mul(t[:], w_sb[nm][kh][:], rstd[kh][:])
                ws[nm].append(t)

        # ---- bias corrections (rank-1 terms vs the UNSCALED weights) ----
        # q bias = bq - (mu*rstd) @ Wq  (per-partition layout over c_out)
        qb_run = []
        for m in range(2):
            ms = slice(m * P, (m + 1) * P)
            pq = ps_mm.tile([P, 1], f32, tag="vec", name="qoffps")
            for kh in range(2):
                nc.tensor.matmul(pq[:], w_sb["q"][kh][:, ms], mrs[kh][:], start=(kh == 0), stop=(kh == 1))
            qb = spool.tile([P, 1], f32, name=f"qb{m}")
            nc.vector.tensor_tensor(out=qb[:], in0=bq_sb[:, m:m + 1], in1=pq[:], op=Alu.subtract)
            qb_run.append(qb)
        # output bias (free layout [1, C]) = bo_folded - (mu*rstd) @ (Wv@Wo)
        voffT = spool.tile([P, 2], f32, name="voffT")
        for m in range(2):
            ms = slice(m * P, (m + 1) * P)
            pv = ps_mm.tile([P, 1], f32, tag="vec", name="voffps")
            for kh in range(2):
                nc.tensor.matmul(pv[:], w_sb["vo"][kh][:, ms], mrs[kh][:], start=(kh == 0), stop=(kh == 1))
            nc.vector.tensor_copy(voffT[:, m:m + 1], pv[:])
        voff_f = spool.tile([1, C], f32, name="voff_f")
        for m in range(2):
            tpv = ps_mm.tile([P, P], f32, tag="vec2", name="voffT_ps")
            nc.tensor.transpose(tpv[:1, :], voffT[:, m:m + 1], ident_sb[:].bitcast(f32))
            nc.vector.tensor_copy(voff_f[:, m * P:(m + 1) * P], tpv[:1, :])
        bo_run = spool.tile([1, C], f32r, name="bo_run")
        nc.vector.tensor_tensor(out=bo_run[:], in0=bo_sb[:], in1=voff_f[:], op=Alu.subtract)

        xf = [x_sb[kh].rearrange("p a b -> p (a b)") for kh in range(2)]

        # ---- k: [c_out m, 2244 tok] image-major ----
        k_sb = [qkv_pool.tile([P, NTOK], f32r, name=f"k{m}") for m in range(2)]
        nch = [(i * 512, min(NTOK, (i + 1) * 512)) for i in range((NTOK + 511) // 512)]
        for m in range(2):
            ms = slice(m * P, (m + 1) * P)
            for a, b in nch:
                pk = ps_mm.tile([P, 512], f32, tag="mm", name="kps")
                for kh in range(2):
                    nc.tensor.matmul(pk[:, :b - a], ws["k"][kh][:, ms],
                                     xf[kh][:, a:b],
                                     start=(kh == 0), stop=(kh == 1))
                any_copy((a // 512) % 2 == 0, k_sb[m][:, a:b], pk[:, :b - a])

        # ---- v' = x @ (rstd * Wv@Wo) + bo: [tok, c_out] token-major ----
        # the output bias rides along as a K=1 rank-1 accumulation (ones row
        # times bo_run), so every token of V' carries the bias exactly once
        ones_col = cpool.tile([1, P], f32r, name="ones_col")
        nc.gpsimd.dma_start(out=ones_col[:], in_=onesd[:].bitcast(f32r))
        NV = (NTOK + P - 1) // P
        v_sb = qkv_pool.tile([P, NV, C], f32r, name="v")
        for t in range(NV):
            tw = min(P, NTOK - t * P)
            pv = ps_mm.tile([P, 512], f32, tag="mm", name="vps")
            for kh in range(2):
                nc.tensor.matmul(pv[:tw, :C], xf[kh][:, t * P:t * P + tw],
                                 ws["vo"][kh][:],
                                 start=(kh == 0), stop=False)
            nc.tensor.matmul(pv[:tw, :C], ones_col[:, :tw],
                             bo_run[:], start=False, stop=True)
            any_copy(t % 2 == 0, v_sb[:tw, t, :], pv[:tw, :C])

        # ---- q: [c_out m, 2048 query tok] with bias ----
        q_sb = [qkv_pool.tile([P, NQ], f32r, name=f"q{m}") for m in range(2)]
        for m in range(2):
            ms = slice(m * P, (m + 1) * P)
            for ci in range(4):
                pq = ps_mm.tile([P, 512], f32, tag="mm", name="qps")
                rhs = [x_sb[kh][:, 1 + 8 * ci:9 + 8 * ci, 1:W + 1] for kh in range(2)]
                for kh in range(2):
                    nc.tensor.matmul(pq[:], ws["q"][kh][:, ms],
                                     rhs[kh], start=(kh == 0), stop=(kh == 1))
                nc.vector.tensor_scalar_add(q_sb[m][:, ci * 512:(ci + 1) * 512], pq[:], qb_run[m][:])

        es_mm.close()
        ps_L = es.enter_context(tc.tile_pool(name="ps_L", bufs=3, space="PSUM"))
        ps_tp = es.enter_context(tc.tile_pool(name="ps_tp", bufs=3, space="PSUM"))
        ps_ctx = es.enter_context(tc.tile_pool(name="ps_ctx", bufs=2, space="PSUM"))

        # ---- attention blocks; E @ V' goes straight to the output ----
        # software-pipelined: L matmul emitted one block ahead so the PE
        # isn't stalled behind the previous block's DVE/ACT stage
        Lps = {}
        def emit_L(bI):
            kt0 = 2 * bI * WP
            Lp = ps_L.tile([P, KT], f32, tag="L", name="Lps")
            for m in range(2):
                nc.tensor.matmul(Lp[:], q_sb[m][:, bI * P:(bI + 1) * P],
                                 k_sb[m][:, kt0:kt0 + KT],
                                 start=(m == 0), stop=(m == 1))
            Lps[bI] = Lp

        emit_L(0)
        emit_L(1)
        for bI in range(NBLK):
            kt0 = 2 * bI * WP          # = (R-1)*66 with R = 1+2*bI
            if bI + 2 < NBLK:
                emit_L(bI + 2)
            Lp = Lps.pop(bI)
            # E columns are laid out from the 128-tile boundary below kt0 so
            # every ctx sub-chunk starts at partition base 0 (PE requirement);
            # the left pad holds exp(-1e30) = 0 and contributes nothing.
            off = kt0 % P
            t0 = kt0 // P
            span = off + KT
            npc = (span + P - 1) // P
            Lm = apool.tile([P, 324], f32, tag="Lm", name="Lm", bufs=3)
            if off:
                nc.vector.memset(Lm[:, :off], NEG)
            nc.vector.tensor_tensor(out=Lm[:, off:off + KT], in0=Lp[:], in1=mask_sb[:], op=Alu.add)
            E = apool.tile([P, 324], f32r, tag="E", name="E", bufs=3)
            Z = apool.tile([P, 1], f32, tag="Z", name="Z", bufs=4)
            nc.scalar.activation(out=E[:, :span], in_=Lm[:, :span], func=Act.Exp,
                                 scale=1.0, accum_out=Z[:])
            rZ = apool.tile([P, 1], f32, tag="rZ", name="rZ", bufs=4)
            nc.vector.reciprocal(rZ[:], Z[:])

            cp = ps_ctx.tile([P, C], f32, tag="ctx", name="ctxps")
            # the three transposed E pieces land in disjoint free stripes of
            # one PSUM tile so a single copy moves them all to SBUF
            tp = ps_tp.tile([P, 3 * P], f32, tag="tp", name="Etps")
            widths = [min(P, span - i * P) for i in range(npc)]
            for i in range(npc):
                nc.tensor.transpose(tp[:widths[i], i * P:(i + 1) * P].bitcast(f32r),
                                    E[:, i * P:i * P + widths[i]],
                                    ident_sb[:])
            Et = apool.tile([P, 3 * P], f32r, tag="Et", name="Et", bufs=3)
            nc.vector.tensor_copy(Et[:], tp[:])
            for i in range(npc):
                nc.tensor.matmul(cp[:], Et[:widths[i], i * P:(i + 1) * P],
                                 v_sb[0:widths[i], t0 + i, :],
                                 start=(i == 0), stop=(i == npc - 1))
            # y = ctx/Z (output bias already folded into V'), token-major
            yt = ypool.tile([P, C], f32, tag="yt", name="yt", bufs=4)
            nc.scalar.activation(out=yt[:], in_=cp[:], func=Act.Copy, scale=rZ[:])
            (nc.sync if bI % 2 == 0 else nc.gpsimd).dma_start(
                out=yd[bI * P:(bI + 1) * P, :], in_=yt[:])

    return nc


def _window_mask():
    m = np.full((P, KT), NEG, dtype=np.float32)
    for r2 in range(2):
        for w in range(W):
            p = r2 * W + w
            for dr in (-1, 0, 1):
                for dw in (-1, 0, 1):
                    j = WP * (r2 + dr + 1) + (w + dw + 1)
                    m[p, j] = 0.0
    return m


def kernel(x, Wq, bq, Wk, bk, Wv, bv, Wo, bo):
    import os
    from concourse import bass_utils

    x = np.asarray(x, dtype=np.float32)
    sc = np.float32(1.0 / np.sqrt(np.float32(C)))
    wq = (np.asarray(Wq) * sc).astype(np.float32)
    bqs = (np.asarray(bq) * sc).astype(np.float32)
    Wo_ = np.asarray(Wo, dtype=np.float32)
    wvo = (np.asarray(Wv, dtype=np.float32) @ Wo_).astype(np.float32)
    bof = (np.asarray(bv, dtype=np.float32) @ Wo_
           + np.asarray(bo, dtype=np.float32)).astype(np.float32)
    mask = _window_mask()
    ident = np.eye(P, dtype=np.float32)

    if "nc" not in _CACHE:
        _CACHE["nc"] = _build_nc()
    nc = _CACHE["nc"]
    if not nc.is_finalized():
        nc.finalize()

    in_maps = []
    for core in range(8):
        b, s = core // 2, core % 2
        xs = np.zeros((C, RW, W), dtype=np.float32)
        pm = np.zeros((P, 2), dtype=np.float32)
        if s == 0:
            xs[:, 1:34, :] = x[b, :, 0:33, :]
            xr = x[b, :, 32:64, :]
            pm[:, 0] = 1.0
        else:
            xs[:, 0:33, :] = x[b, :, 31:64, :]
            xr = x[b, :, 0:32, :]
            pm[:, 1] = 1.0
        in_maps.append({
            "x": xs, "xr": np.ascontiguousarray(xr).astype(ml_dtypes.bfloat16),
            "wq": wq, "wk": np.asarray(Wk, dtype=np.float32), "wvo": wvo,
            "bq": bqs, "bo": bof, "mask": mask, "ident": ident, "pm": pm,
            "ones": np.ones((1, P), dtype=np.float32),
        })

    trace = bool(os.environ.get("KPROF"))
    import time as _time
    t0 = _time.monotonic()
    res = bass_utils.run_bass_kernel_spmd(nc, in_maps, core_ids=list(range(8)),
                                          trace=trace)
    _CACHE["run_s"] = _time.monotonic() - t0
    _CACHE["last"] = res
    y = np.empty((B, C, H, W), dtype=np.float32)
    for core in range(8):
        b, s = core // 2, core % 2
        y[b, :, s * HS:(s + 1) * HS, :] = (
            res.results[core]["y"].reshape(HS, W, C).transpose(2, 0, 1))
    return y


if __name__ == "__main__":
    rng = np.random.default_rng(0)
    ins = {
        "x": rng.standard_normal((B, C, H, W), dtype=np.float32),
        "Wq": rng.standard_normal((C, C), dtype=np.float32) / 16,
        "bq": rng.standard_normal(C, dtype=np.float32) * 0.02,
        "Wk": rng.standard_normal((C, C), dtype=np.float32) / 16,
        "bk": rng.standard_normal(C, dtype=np.float32) * 0.02,
        "Wv": rng.standard_normal((C, C), dtype=np.float32) / 16,
        "bv": rng.standard_normal(C, dtype=np.float32) * 0.02,
        "Wo": rng.standard_normal((C, C), dtype=np.float32) / 16,
        "bo": rng.standard_normal(C, dtype=np.float32) * 0.02,
    }
    out = kernel(**ins)
    print(out.shape, out.dtype, np.abs(out).mean())


# revision 42
# speedup vs baseline: 1.0859x; 1.0859x over previous
"""Trainium2 Bass kernel: 3x3 windowed 2D attention layer (LN -> QKV -> win-attn -> out proj).

Sharding: 8 cores = (batch b, H-half s). Each core handles one batch image's
32 query rows with a 1-row halo. LayerNorm is folded into the weights:
  xn = (x - mu) * rstd  (per (b,c) over full HxW)
  q/k/v = xn @ W + b  ==  x @ (rstd*W) - (mu*rstd) @ W + b
The k offset/bias are softmax-invariant (constant shift of every key seen by
a query), the v offset/bias fold into the output bias, and the output
projection folds into V entirely (V' = x @ (rstd * (Wv@Wo)) since row
scaling commutes through left matmul). Only q keeps a small rank-1 runtime
correction. Stats need the full image, so each core also loads the other
half's rows (xr) just for stats.

Attention: per 2-query-row block, logits[128 tok, 264 keys] = one PE matmul
(q block vs 4 key rows in a 66-wide zero-padded layout). An additive mask
(-1e30 off the 9 window diagonals) + ACT exp(accum=Z) implement the 9-way
softmax over the 264-wide row (max subtraction is skipped: |logit| <= ~8 for
standardized inputs). ctx@Wo comes from a PE matmul of transposed E chunks
against token-major V' = x@(rstd*Wv@Wo); output is written token-major and
transposed on the host during the gather.
"""

import ml_dtypes
import numpy as np

P = 128
C = 256
B, H, W = 4, 64, 64
HS = 32          # query rows per core
RW = 34          # rows with halo (1 pad/real row above + below)
WP = W + 2       # zero-padded width
NTOK = RW * WP   # 2244 tokens per core (padded image)
NQ = HS * W      # 2048 query tokens
NBLK = 16        # query row-pair blocks
KT = 4 * WP      # 264 keys per block (4 rows x 66)
NEG = -1.0e30

_CACHE = {}


def _build_nc():
    import concourse.bass as bass  # noqa: F401
    import concourse.mybir as mybir
    import concourse.tile as tile
    from concourse import bacc
    from contextlib import ExitStack

    f32 = mybir.dt.float32
    f32r = mybir.dt.float32r
    Alu = mybir.AluOpType
    Act = mybir.ActivationFunctionType

    nc = bacc.Bacc(None, target_bir_lowering=False, debug=False)

    xd = nc.declare_dram_parameter("x", [C, RW, W], f32, isOutput=False)
    xrd = nc.declare_dram_parameter("xr", [C, HS, W], mybir.dt.bfloat16, isOutput=False)
    wqd = nc.declare_dram_parameter("wq", [C, C], f32, isOutput=False)
    wkd = nc.declare_dram_parameter("wk", [C, C], f32, isOutput=False)
    wvod = nc.declare_dram_parameter("wvo", [C, C], f32, isOutput=False)
    bqd = nc.declare_dram_parameter("bq", [C], f32, isOutput=False)
    bod = nc.declare_dram_parameter("bo", [C], f32, isOutput=False)
    maskd = nc.declare_dram_parameter("mask", [P, KT], f32, isOutput=False)
    pmd = nc.declare_dram_parameter("pm", [P, 2], f32, isOutput=False)
    identd = nc.declare_dram_parameter("ident", [P, P], f32, isOutput=False)
    onesd = nc.declare_dram_parameter("ones", [1, P], f32, isOutput=False)
    yd = nc.declare_dram_parameter("y", [NQ, C], f32, isOutput=True)

    with tile.TileContext(nc) as tc, ExitStack() as es:
        cpool = es.enter_context(tc.tile_pool(name="const", bufs=1))
        xpool = es.enter_context(tc.tile_pool(name="x", bufs=1))
        spool = es.enter_context(tc.tile_pool(name="stat", bufs=1))
        scrpool = es.enter_context(tc.tile_pool(name="scr", bufs=2))
        qkv_pool = es.enter_context(tc.tile_pool(name="qkv", bufs=1))
        apool = es.enter_context(tc.tile_pool(name="attn", bufs=3))
        ypool = es.enter_context(tc.tile_pool(name="y", bufs=3))
        es_mm = ExitStack()
        ps_mm = es_mm.enter_context(tc.tile_pool(name="ps_mm", bufs=4, space="PSUM"))

        def any_copy(on_dve, out, in_):
            if on_dve:
                nc.vector.tensor_copy(out, in_)
            else:
                nc.scalar.activation(out=out, in_=in_, func=Act.Copy)

        # ---- load x (into 66-wide zero-padded layout) ----
        x_sb = []
        for kh in range(2):
            xt = xpool.tile([P, RW, WP], f32r, name=f"x{kh}")
            nc.sync.dma_start(out=xt[:, 0:17, 1:W + 1], in_=xd[kh * P:(kh + 1) * P, 0:17, :].bitcast(f32r))
            nc.scalar.dma_start(out=xt[:, 17:RW, 1:W + 1], in_=xd[kh * P:(kh + 1) * P, 17:RW, :].bitcast(f32r))
            x_sb.append(xt)
        xr_sb = []
        for kh in range(2):
            xrt = xpool.tile([P, HS, W], mybir.dt.bfloat16, name=f"xr{kh}")
            nc.sync.dma_start(out=xrt[:, 0:16, :], in_=xrd[kh * P:(kh + 1) * P, 0:16, :])
            nc.scalar.dma_start(out=xrt[:, 16:HS, :], in_=xrd[kh * P:(kh + 1) * P, 16:HS, :])
            xr_sb.append(xrt)

        # ---- constants ----
        w_sb = {}
        for nm, d in (("q", wqd), ("k", wkd), ("vo", wvod)):
            w_sb[nm] = [cpool.tile([P, C], f32, name=f"w{nm}{kh}") for kh in range(2)]
            for kh in range(2):
                nc.gpsimd.dma_start(out=w_sb[nm][kh][:], in_=d[kh * P:(kh + 1) * P, :])
        mask_sb = cpool.tile([P, KT], f32, name="mask")
        nc.gpsimd.dma_start(out=mask_sb[:], in_=maskd[:])
        ident_sb = cpool.tile([P, P], f32r, name="ident")
        nc.gpsimd.dma_start(out=ident_sb[:], in_=identd[:].bitcast(f32r))
        bq_sb = cpool.tile([P, 2], f32, name="bq")
        nc.gpsimd.dma_start(out=bq_sb[:], in_=bqd.rearrange("(h p) -> p h", p=P))
        bo_sb = cpool.tile([1, C], f32, name="bo")
        nc.gpsimd.dma_start(out=bo_sb[:], in_=bod.rearrange("(a c) -> a c", a=1))
        pm_sb = cpool.tile([P, 2], f32, name="pm")
        nc.gpsimd.dma_start(out=pm_sb[:], in_=pmd[:])

        # ---- layernorm stats over the full image ----
        # sum of squares on ACT (Square + accumulate), sums on DVE, in parallel
        rstd, mrs = [], []
        eps_t = spool.tile([P, 1], f32, name="eps")
        nc.vector.memset(eps_t[:], 1e-5)
        for kh in range(2):
            # own-half partial sums per DMA chunk (overlaps the x load);
            # full-image stats come from a pair-wise AllReduce below
            regs = [x_sb[kh][:, 1:17, 1:W + 1].bitcast(f32),
                    x_sb[kh][:, 17:HS + 1, 1:W + 1].bitcast(f32),
                    xr_sb[kh][:, 0:16, :], xr_sb[kh][:, 16:HS, :]]
            sq_parts, sm_parts = [], []
            for pi, rg in enumerate(regs):
                scr = scrpool.tile([P, 16, W], f32, tag="scr", name=f"scr{kh}{pi}")
                sqp = spool.tile([P, 1], f32, name=f"ssqp{kh}{pi}")
                nc.scalar.activation(out=scr[:], in_=rg, func=Act.Square, accum_out=sqp[:])
                sq_parts.append(sqp)
                smp = spool.tile([P, 1], f32, name=f"smp{kh}{pi}")
                nc.vector.tensor_reduce(out=smp[:], in_=rg, axis=mybir.AxisListType.XY, op=Alu.add)
                sm_parts.append(smp)
            ssq = spool.tile([P, 1], f32, name=f"ssq{kh}")
            nc.vector.tensor_tensor(out=ssq[:], in0=sq_parts[0][:], in1=sq_parts[1][:], op=Alu.add)
            nc.vector.tensor_tensor(out=ssq[:], in0=ssq[:], in1=sq_parts[2][:], op=Alu.add)
            nc.vector.tensor_tensor(out=ssq[:], in0=ssq[:], in1=sq_parts[3][:], op=Alu.add)
            sm = spool.tile([P, 1], f32, name=f"sm{kh}")
            nc.vector.tensor_tensor(out=sm[:], in0=sm_parts[0][:], in1=sm_parts[1][:], op=Alu.add)
            nc.vector.tensor_tensor(out=sm[:], in0=sm[:], in1=sm_parts[2][:], op=Alu.add)
            nc.vector.tensor_tensor(out=sm[:], in0=sm[:], in1=sm_parts[3][:], op=Alu.add)
            mean = spool.tile([P, 1], f32, name=f"mean{kh}")
            nc.vector.tensor_scalar_mul(mean[:], sm[:], 1.0 / (H * W))
            ex2 = spool.tile([P, 1], f32, name=f"ex2{kh}")
            nc.vector.tensor_scalar_mul(ex2[:], ssq[:], 1.0 / (H * W))
            msq = spool.tile([P, 1], f32, name=f"msq{kh}")
            nc.vector.tensor_tensor(out=msq[:], in0=mean[:], in1=mean[:], op=Alu.mult)
            var = spool.tile([P, 1], f32, name=f"var{kh}")
            nc.vector.tensor_tensor(out=var[:], in0=ex2[:], in1=msq[:], op=Alu.subtract)
            std = spool.tile([P, 1], f32, name=f"std{kh}")
            nc.scalar.activation(out=std[:], in_=var[:], func=Act.Sqrt, bias=eps_t[:], scale=1.0)
            rs = spool.tile([P, 1], f32, name=f"rstd{kh}")
            nc.vector.reciprocal(rs[:], std[:])
            rstd.append(rs)
            mr = spool.tile([P, 1], f32, name=f"mrs{kh}")
            nc.vector.tensor_tensor(out=mr[:], in0=mean[:], in1=rs[:], op=Alu.mult)
            mrs.append(mr)

            # Fill pad positions of x with the channel mean so pad tokens
            # behave exactly like xn=0 under the rstd-folded weights (keeps
            # the key offset uniform across a window -> softmax-invariant).
            # The pad row location differs per core; pm selects it (0/1).
            for ri, r in ((0, 0), (1, RW - 1)):
                br = spool.tile([P, 1], f32, name=f"br{kh}{ri}")
                nc.vector.tensor_tensor(out=br[:], in0=mean[:], in1=pm_sb[:, ri:ri + 1], op=Alu.mult)
                nc.scalar.activation(out=x_sb[kh][:, r, :], in_=x_sb[kh][:, r, :].bitcast(f32),
                                     func=Act.Identity, bias=br[:], scale=1.0)
            for c0 in (0, WP - 1):
                nc.scalar.activation(out=x_sb[kh][:, :, c0:c0 + 1],
                                     in_=x_sb[kh][:, :, c0:c0 + 1].bitcast(f32),
                                     func=Act.Identity, bias=mean[:], scale=0.0)

        # ---- scale W rows by rstd:  W'[c, :] = rstd_c * W[c, :] ----
        ws = {}
        for nm in ("q", "k", "vo"):
            ws[nm] = []
            for kh in range(2):
                t = cpool.tile([P, C], f32r, name=f"ws{nm}{kh}")
                nc.vector.tensor_scalar_mul(t[:], w_sb[nm][kh][:], rstd[kh][:])
                ws[nm].append(t)

        # ---- bias corrections (rank-1 terms vs the UNSCALED weights) ----
        # q bias = bq - (mu*rstd) @ Wq  (per-partition layout over c_out)
        qb_run = []
        for m in range(2):
            ms = slice(m * P, (m + 1) * P)
            pq = ps_mm.tile([P, 1], f32, tag="vec", name="qoffps", bufs=2)
            for kh in range(2):
                nc.tensor.matmul(pq[:], w_sb["q"][kh][:, ms], mrs[kh][:], start=(kh == 0), stop=(kh == 1))
            qb = spool.tile([P, 1], f32, name=f"qb{m}")
            nc.vector.tensor_tensor(out=qb[:], in0=bq_sb[:, m:m + 1], in1=pq[:], op=Alu.subtract)
            qb_run.append(qb)
        # output bias (free layout [1, C]) = bo_folded - (mu*rstd) @ (Wv@Wo)
        voffT = spool.tile([P, 2], f32, name="voffT")
        for m in range(2):
            ms = slice(m * P, (m + 1) * P)
            pv = ps_mm.tile([P, 1], f32, tag="vec", name="voffps", bufs=2)
            for kh in range(2):
                nc.tensor.matmul(pv[:], w_sb["vo"][kh][:, ms], mrs[kh][:], start=(kh == 0), stop=(kh == 1))
            nc.vector.tensor_copy(voffT[:, m:m + 1], pv[:])
        voff_f = spool.tile([1, C], f32, name="voff_f")
        for m in range(2):
            tpv = ps_mm.tile([P, P], f32, tag="vec2", name="voffT_ps", bufs=2)
            nc.tensor.transpose(tpv[:1, :], voffT[:, m:m + 1], ident_sb[:].bitcast(f32))
            nc.vector.tensor_copy(voff_f[:, m * P:(m + 1) * P], tpv[:1, :])
        bo_run = spool.tile([1, C], f32r, name="bo_run")
        nc.vector.tensor_tensor(out=bo_run[:], in0=bo_sb[:], in1=voff_f[:], op=Alu.subtract)

        xf = [x_sb[kh].rearrange("p a b -> p (a b)") for kh in range(2)]

        # ---- k: [c_out m, 2244 tok] image-major ----
        k_sb = [qkv_pool.tile([P, NTOK], f32r, name=f"k{m}") for m in range(2)]
        nch = [(i * 512, min(NTOK, (i + 1) * 512)) for i in range((NTOK + 511) // 512)]
        for m in range(2):
            ms = slice(m * P, (m + 1) * P)
            for a, b in nch:
                pk = ps_mm.tile([P, 512], f32, tag="mm", name="kps")
                for kh in range(2):
                    nc.tensor.matmul(pk[:, :b - a], ws["k"][kh][:, ms],
                                     xf[kh][:, a:b],
                                     start=(kh == 0), stop=(kh == 1))
                any_copy((a // 512) % 2 == 0, k_sb[m][:, a:b], pk[:, :b - a])

        # ---- v' = x @ (rstd * Wv@Wo) + bo: [tok, c_out] token-major ----
        # the output bias rides along as a K=1 rank-1 accumulation (ones row
        # times bo_run), so every token of V' carries the bias exactly once
        ones_col = cpool.tile([1, P], f32r, name="ones_col")
        nc.gpsimd.dma_start(out=ones_col[:], in_=onesd[:].bitcast(f32r))
        NV = (NTOK + P - 1) // P
        v_sb = qkv_pool.tile([P, NV, C], f32r, name="v")
        for t in range(NV):
            tw = min(P, NTOK - t * P)
            pv = ps_mm.tile([P, 512], f32, tag="mm", name="vps")
            for kh in range(2):
                nc.tensor.matmul(pv[:tw, :C], xf[kh][:, t * P:t * P + tw],
                                 ws["vo"][kh][:],
                                 start=(kh == 0), stop=False)
            nc.tensor.matmul(pv[:tw, :C], ones_col[:, :tw],
                             bo_run[:], start=False, stop=True)
            any_copy(t % 2 == 0, v_sb[:tw, t, :], pv[:tw, :C])

        # ---- q: [c_out m, 2048 query tok] with bias ----
        q_sb = [qkv_pool.tile([P, NQ], f32r, name=f"q{m}") for m in range(2)]
        for m in range(2):
            ms = slice(m * P, (m + 1) * P)
            for ci in range(4):
                pq = ps_mm.tile([P, 512], f32, tag="mm", name="qps")
                rhs = [x_sb[kh][:, 1 + 8 * ci:9 + 8 * ci, 1:W + 1] for kh in range(2)]
                for kh in range(2):
                    nc.tensor.matmul(pq[:], ws["q"][kh][:, ms],
                                     rhs[kh], start=(kh == 0), stop=(kh == 1))
                nc.vector.tensor_scalar_add(q_sb[m][:, ci * 512:(ci + 1) * 512], pq[:], qb_run[m][:])

        es_mm.close()
        ps_L = es.enter_context(tc.tile_pool(name="ps_L", bufs=3, space="PSUM"))
        ps_tp = es.enter_context(tc.tile_pool(name="ps_tp", bufs=2, space="PSUM"))
        ps_ctx = es.enter_context(tc.tile_pool(name="ps_ctx", bufs=3, space="PSUM"))

        # ---- attention blocks; E @ V' goes straight to the output ----
        # software-pipelined: L matmul emitted one block ahead so the PE
        # isn't stalled behind the previous block's DVE/ACT stage
        Lps = {}
        def emit_L(bI):
            kt0 = 2 * bI * WP
            Lp = ps_L.tile([P, KT], f32, tag="L", name="Lps")
            for m in range(2):
                nc.tensor.matmul(Lp[:], q_sb[m][:, bI * P:(bI + 1) * P],
                                 k_sb[m][:, kt0:kt0 + KT],
                                 start=(m == 0), stop=(m == 1))
            Lps[bI] = Lp

        emit_L(0)
        emit_L(1)
        for bI in range(NBLK):
            kt0 = 2 * bI * WP          # = (R-1)*66 with R = 1+2*bI
            if bI + 2 < NBLK:
                emit_L(bI + 2)
            Lp = Lps.pop(bI)
            # E columns are laid out from the 128-tile boundary below kt0 so
            # every ctx sub-chunk starts at partition base 0 (PE requirement);
            # the left pad holds exp(-1e30) = 0 and contributes nothing.
            off = kt0 % P
            t0 = kt0 // P
            span = off + KT
            npc = (span + P - 1) // P
            Lm = apool.tile([P, 324], f32, tag="Lm", name="Lm", bufs=3)
            if off:
                nc.vector.memset(Lm[:, :off], NEG)
            nc.vector.tensor_tensor(out=Lm[:, off:off + KT], in0=Lp[:], in1=mask_sb[:], op=Alu.add)
            E = apool.tile([P, 324], f32r, tag="E", name="E", bufs=3)
            Z = apool.tile([P, 1], f32, tag="Z", name="Z", bufs=4)
            nc.scalar.activation(out=E[:, :span], in_=Lm[:, :span], func=Act.Exp,
                                 scale=1.0, accum_out=Z[:])
            rZ = apool.tile([P, 1], f32, tag="rZ", name="rZ", bufs=4)
            nc.vector.reciprocal(rZ[:], Z[:])

            cp = ps_ctx.tile([P, C], f32, tag="ctx", name="ctxps")
            # the three transposed E pieces land in disjoint free stripes of
            # one PSUM tile so a single copy moves them all to SBUF
            tp = ps_tp.tile([P, 3 * P], f32, tag="tp", name="Etps")
            widths = [min(P, span - i * P) for i in range(npc)]
            for i in range(npc):
                nc.tensor.transpose(tp[:widths[i], i * P:(i + 1) * P].bitcast(f32r),
                                    E[:, i * P:i * P + widths[i]],
                                    ident_sb[:])
            Et = apool.tile([P, 3 * P], f32r, tag="Et", name="Et", bufs=3)
            nc.vector.tensor_copy(Et[:], tp[:])
            for i in range(npc):
                nc.tensor.matmul(cp[:], Et[:widths[i], i * P:(i + 1) * P],
                                 v_sb[0:widths[i], t0 + i, :],
                                 start=(i == 0), stop=(i == npc - 1))
            # y = ctx/Z (output bias already folded into V'), token-major
            yt = ypool.tile([P, C], f32, tag="yt", name="yt", bufs=4)
            nc.scalar.activation(out=yt[:], in_=cp[:], func=Act.Copy, scale=rZ[:])
            (nc.sync if bI % 2 == 0 else nc.gpsimd).dma_start(
                out=yd[bI * P:(bI + 1) * P, :], in_=yt[:])

    return nc


def _window_mask():
    m = np.full((P, KT), NEG, dtype=np.float32)
    for r2 in range(2):
        for w in range(W):
            p = r2 * W + w
            for dr in (-1, 0, 1):
                for dw in (-1, 0, 1):
                    j = WP * (r2 + dr + 1) + (w + dw + 1)
                    m[p, j] = 0.0
    return m


def kernel(x, Wq, bq, Wk, bk, Wv, bv, Wo, bo):
    import os
    from concourse import bass_utils

    x = np.asarray(x, dtype=np.float32)
    sc = np.float32(1.0 / np.sqrt(np.float32(C)))
    wq = (np.asarray(Wq) * sc).astype(np.float32)
    bqs = (np.asarray(bq) * sc).astype(np.float32)
    Wo_ = np.asarray(Wo, dtype=np.float32)
    wvo = (np.asarray(Wv, dtype=np.float32) @ Wo_).astype(np.float32)
    bof = (np.asarray(bv, dtype=np.float32) @ Wo_
           + np.asarray(bo, dtype=np.float32)).astype(np.float32)
    mask = _window_mask()
    ident = np.eye(P, dtype=np.float32)

    if "nc" not in _CACHE:
        _CACHE["nc"] = _build_nc()
    nc = _CACHE["nc"]
    if not nc.is_finalized():
        nc.finalize()

    in_maps = []
    for core in range(8):
        b, s = core // 2, core % 2
        xs = np.zeros((C, RW, W), dtype=np.float32)
        pm = np.zeros((P, 2), dtype=np.float32)
        if s == 0:
            xs[:, 1:34, :] = x[b, :, 0:33, :]
            xr = x[b, :, 32:64, :]
            pm[:, 0] = 1.0
        else:
            xs[:, 0:33, :] = x[b, :, 31:64, :]
            xr = x[b, :, 0:32, :]
            pm[:, 1] = 1.0
        in_maps.append({
            "x": xs, "xr": np.ascontiguousarray(xr).astype(ml_dtypes.bfloat16),
            "wq": wq, "wk": np.asarray(Wk, dtype=np.float32), "wvo": wvo,
            "bq": bqs, "bo": bof, "mask": mask, "ident": ident, "pm": pm,
            "ones": np.ones((1, P), dtype=np.float32),
        })

    trace = bool(os.environ.get("KPROF"))
    import time as _time
    t0 = _time.monotonic()
    res = bass_utils.run_bass_kernel_spmd(nc, in_maps, core_ids=list(range(8)),
                                          trace=trace)
    _CACHE["run_s"] = _time.monotonic() - t0
    _CACHE["last"] = res
    y = np.empty((B, C, H, W), dtype=np.float32)
    for core in range(8):
        b, s = core // 2, core % 2
        y[b, :, s * HS:(s + 1) * HS, :] = (
            res.results[core]["y"].reshape(HS, W, C).transpose(2, 0, 1))
    return y


if __name__ == "__main__":
    rng = np.random.default_rng(0)
    ins = {
        "x": rng.standard_normal((B, C, H, W), dtype=np.float32),
        "Wq": rng.standard_normal((C, C), dtype=np.float32) / 16,
        "bq": rng.standard_normal(C, dtype=np.float32) * 0.02,
        "Wk": rng.standard_normal((C, C), dtype=np.float32) / 16,
        "bk": rng.standard_normal(C, dtype=np.float32) * 0.02,
        "Wv": rng.standard_normal((C, C), dtype=np.float32) / 16,
        "bv": rng.standard_normal(C, dtype=np.float32) * 0.02,
        "Wo": rng.standard_normal((C, C), dtype=np.float32) / 16,
        "bo": rng.standard_normal(C, dtype=np.float32) * 0.02,
    }
    out = kernel(**ins)
    print(out.shape, out.dtype, np.abs(out).mean())
